# revision 11
# baseline (speedup 1.0000x reference)
"""Trainium2 Bass kernel for nn_Attention (2-batch, 16-head, n=2048, d=64 causal
attention with LayerNorm-projected l2-normalized q/k, relative position bias,
and output projection), SPMD across 8 NeuronCores.

Sharding: launch A tensor-parallels the 16 heads (2 heads per core, both
batches on every core) and emits normalized, transposed attention outputs;
launch B row-shards the final @ Wo matmul across the 8 cores ((batch, i-chunk)
per core).

Launch A design notes (v2 -- tuned to keep the PE clock-gate warm):
- Weights are gamma-folded AND column-centered on the host, so the LayerNorm
  mean subtraction is exact with no rank-1 correction matmuls on device.
- LN rstd cancels in the q/k l2norm.  For v it is folded into the softmax:
  E' = exp(sim + bias + ln rstd_j), and the denominator column of the
  v-extension carries std_j instead of 1, so no [128 x 512] rstd broadcast is
  ever materialized; only [1, 512] row stats + tiny PE transposes.
- rel_pos_bias is pre-transposed, causal-masked (-1e30) and bf16 on the host.
  On device it is injected into PSUM by an identity matmul (start=True) and
  the q@k matmul accumulates onto it (stop=True) -- no vector-engine add; the
  exp reads PSUM directly.  This keeps the tensor-engine stream dense, which
  keeps the HAM clock-gate at 2.4 GHz (the baseline ran all of phase 2 at
  1.2 GHz).
- Fully-masked columns of diagonal tiles are skipped (sub-ranged matmuls/exp).
- Softmax normalization happens in launch A at PSUM-evacuation time (row 64 of
  the attn@v accumulator is the denominator), so launch B is a pure bf16
  matmul.
- phase 1 (chunk projections) and phase 2 (chunk attention) are interleaved
  per i-chunk in one instruction stream; attn@v is software-pipelined one tile
  behind the sim matmuls so the PE never waits on the scalar-engine exp.
"""

import numpy as np

HEADS = 16
DH = 64
B = 2
N = 2048
DIM = 1024
EH = 128          # per-core slice of the inner dim (2 heads x 64)
NCORES = 8
IC = 512          # i-chunk width
NIC = N // IC     # 4 i-chunks
JT = 128          # j-tile width
NJT = N // JT     # 16 j-tiles
NCT = DIM // 128  # 8 contraction tiles
LN_EPS = 1e-5
NEG = -1e30

_cache = {}


def _build_launch_a():
    import concourse.bass as bass
    import concourse.tile as tile
    from concourse import bacc, mybir
    from concourse.masks import make_identity

    F32 = mybir.dt.float32
    F32R = mybir.dt.float32r
    BF16 = mybir.dt.bfloat16
    AF = mybir.ActivationFunctionType
    ALU = mybir.AluOpType
    nc = bacc.Bacc(None)
    xT_d = nc.declare_dram_parameter("xT", [B, DIM, N], BF16, isOutput=False)
    rpbT_d = nc.declare_dram_parameter("rpbT", [2, N, N], BF16, isOutput=False)
    wq_d = nc.declare_dram_parameter("wq", [DIM, EH], BF16, isOutput=False)
    wk_d = nc.declare_dram_parameter("wk", [DIM, EH], BF16, isOutput=False)
    wv_d = nc.declare_dram_parameter("wv", [DIM, EH], BF16, isOutput=False)
    qs2_d = nc.declare_dram_parameter("qs2", [EH], F32, isOutput=False)
    ks2_d = nc.declare_dram_parameter("ks2", [EH], F32, isOutput=False)
    kb_d = nc.declare_dram_parameter("kb", [B, N], F32, isOutput=False)
    at_d = nc.declare_dram_parameter("at_out", [B, EH, N], BF16, isOutput=True)

    with tile.TileContext(nc) as tc:
        import contextlib
        with contextlib.ExitStack() as ctx:
            pers = ctx.enter_context(tc.tile_pool(name="pers", bufs=1))
            # SBUF pools
            xrp = ctx.enter_context(tc.tile_pool(name="xrp", bufs=3))
            rp_pool = ctx.enter_context(tc.tile_pool(name="rp_pool", bufs=3))
            x2p = ctx.enter_context(tc.tile_pool(name="x2p", bufs=3))
            rowp = ctx.enter_context(tc.tile_pool(name="rowp", bufs=2))
            sqp = ctx.enter_context(tc.tile_pool(name="sqp", bufs=2))
            srp = ctx.enter_context(tc.tile_pool(name="srp", bufs=2))
            ep = ctx.enter_context(tc.tile_pool(name="ep", bufs=4))
            atp = ctx.enter_context(tc.tile_pool(name="atp", bufs=2))
            # PSUM pools: tl(4) + sp(2) + av(2) = 8 banks
            tl_ps = ctx.enter_context(tc.tile_pool(name="tl_ps", bufs=4, space="PSUM"))
            sp_ps = ctx.enter_context(tc.tile_pool(name="sp_ps", bufs=2, space="PSUM"))
            av_ps = ctx.enter_context(tc.tile_pool(name="av_ps", bufs=2, space="PSUM"))

            # ---------- constants ----------
            onescol_f = pers.tile([128, 1], F32, tag="onescol_f")
            nc.vector.memset(onescol_f, 1.0)
            onescol_bf = pers.tile([128, 1], BF16, tag="onescol_bf")
            nc.vector.tensor_copy(out=onescol_bf, in_=onescol_f)
            row_f = pers.tile([1, 512], F32, tag="row_f")
            nc.vector.memset(row_f, 1.0)
            warm_row = pers.tile([1, 512], BF16, tag="warm_row")
            nc.vector.tensor_copy(out=warm_row, in_=row_f)
            ones64_r = pers.tile([1, 64], F32R, tag="ones64_r")
            nc.vector.tensor_copy(out=ones64_r, in_=row_f[:, 0:64])
            o2_f = pers.tile([128, 2], F32, tag="o2_f")
            nc.vector.memset(o2_f, 0.0)
            nc.vector.memset(o2_f[0:64, 0:1], 1.0)
            nc.vector.memset(o2_f[64:128, 1:2], 1.0)
            ones2blk = pers.tile([128, 2], F32R, tag="ones2blk")
            nc.vector.tensor_copy(out=ones2blk, in_=o2_f)
            ident = pers.tile([128, 128], F32, tag="ident")
            make_identity(nc, ident)
            ident_bf = pers.tile([128, 128], BF16, tag="ident_bf")
            nc.vector.tensor_copy(out=ident_bf, in_=ident)
            eps1 = pers.tile([1, 1], F32, tag="eps1")
            nc.vector.memset(eps1, LN_EPS)
            eps2 = pers.tile([2, 1], F32, tag="eps2")
            nc.vector.memset(eps2, 1e-24)

            # ---- PE warm-up: dummy matmuls while the first DMAs stream ----
            warm_ps = av_ps.tile([1, IC], F32, tag="av")
            for _ in range(20):
                nc.tensor.matmul(warm_ps, onescol_bf[0:1, :], warm_row,
                                 start=True, stop=True)

            # scale rows -> block-diag [2, 128] (qs2blk[h, e] = qs2[e] iff head(e)==h)
            qsb_f = pers.tile([2, 128], F32, tag="qsb_f")
            nc.vector.memset(qsb_f, 0.0)
            nc.sync.dma_start(out=qsb_f[0:1, 0:64], in_=qs2_d.ap()[0:64].unsqueeze(0))
            nc.sync.dma_start(out=qsb_f[1:2, 64:128], in_=qs2_d.ap()[64:128].unsqueeze(0))
            qs2blk = pers.tile([2, 128], F32R, tag="qs2blk")
            nc.vector.tensor_copy(out=qs2blk, in_=qsb_f)
            ksb_f = pers.tile([2, 128], F32, tag="ksb_f")
            nc.vector.memset(ksb_f, 0.0)
            nc.sync.dma_start(out=ksb_f[0:1, 0:64], in_=ks2_d.ap()[0:64].unsqueeze(0))
            nc.sync.dma_start(out=ksb_f[1:2, 64:128], in_=ks2_d.ap()[64:128].unsqueeze(0))
            ks2blk = pers.tile([2, 128], F32R, tag="ks2blk")
            nc.vector.tensor_copy(out=ks2blk, in_=ksb_f)

            kbT = pers.tile([128, B, NJT], F32, tag="kbT")
            nc.sync.dma_start(out=kbT, in_=kb_d.ap().rearrange("b (t p) -> p b t", p=128))

            # ---------- weights: already gamma-folded + centered (host) ----------
            wps = {}
            for nm, wd in (("q", wq_d), ("k", wk_d), ("v", wv_d)):
                wp = pers.tile([128, NCT, EH], BF16, tag=f"w{nm}p")
                nc.sync.dma_start(out=wp, in_=wd.ap().rearrange("(t p) e -> p t e", p=128))
                wps[nm] = wp

            # ---------- persistent per-batch products ----------
            qhat = [pers.tile([128, N], BF16, tag=f"qhat{b}", name=f"qhat{b}") for b in range(B)]
            khat = [pers.tile([128, N], BF16, tag=f"khat{b}", name=f"khat{b}") for b in range(B)]
            v_all = [pers.tile([128, NJT, 130], BF16, tag=f"vall{b}", name=f"vall{b}") for b in range(B)]
            ebias = pers.tile([128, B, NJT], F32, tag="ebias")

            for ic in range(NIC):
                isl = slice(ic * IC, (ic + 1) * IC)
                jmax = (IC // 128) * (ic + 1)

                # ---- prefetch rel-pos bias tiles for this i-chunk ----
                rps = []
                for h in range(2):
                    rp = rp_pool.tile([128, NJT, IC], BF16, tag="rp", name=f"rp{ic}{h}")
                    nfull = jmax - 4
                    if nfull > 0:
                        nc.sync.dma_start(
                            out=rp[:, 0:nfull, :],
                            in_=rpbT_d.ap()[h, 0:nfull * 128, isl].rearrange(
                                "(t p) i -> p t i", p=128))
                    for k in range(4):
                        jt = jmax - 4 + k
                        off = 128 * k
                        nc.sync.dma_start(
                            out=rp[:, jt, off:IC],
                            in_=rpbT_d.ap()[h, jt * 128:(jt + 1) * 128,
                                            ic * IC + off:(ic + 1) * IC])
                    rps.append(rp)

                # ================= phase 1 chunk: LN stats + projections =================
                for b in range(B):
                    xr = xrp.tile([128, NCT, IC], BF16, tag="xr")
                    for half in range(2):
                        hs = slice(half * (NCT // 2), (half + 1) * (NCT // 2))
                        nc.sync.dma_start(
                            out=xr[:, hs, :],
                            in_=xT_d.ap()[b, half * 512:(half + 1) * 512, isl].rearrange(
                                "(t p) n -> p t n", p=128))

                    # --- stats (rows, [1, IC]) ---
                    sx = sp_ps.tile([1, IC], F32, tag="sp")
                    for ct in range(NCT):
                        nc.tensor.matmul(sx, onescol_bf, xr[:, ct, :],
                                         start=(ct == 0), stop=(ct == NCT - 1))
                    sxx = sp_ps.tile([1, IC], F32, tag="sp")
                    for ct in range(NCT):
                        x2 = x2p.tile([128, IC], BF16, tag="x2")
                        nc.vector.tensor_mul(x2, xr[:, ct, :], xr[:, ct, :])
                        nc.tensor.matmul(sxx, onescol_bf, x2,
                                         start=(ct == 0), stop=(ct == NCT - 1))
                    mu2 = rowp.tile([1, IC], F32, tag="row")
                    nc.scalar.activation(out=mu2, in_=sx, func=AF.Square, scale=1.0 / DIM)
                    # var = sxx/DIM - mu2
                    var = rowp.tile([1, IC], F32, tag="row")
                    nc.vector.scalar_tensor_tensor(
                        out=var, in0=sxx, scalar=1.0 / DIM, in1=mu2,
                        op0=ALU.mult, op1=ALU.subtract)
                    # rows: lnr = ln(rstd) = -0.5*ln(var+eps), std = sqrt(var+eps)
                    lnv = rowp.tile([1, IC], F32, tag="row")
                    nc.scalar.activation(out=lnv, in_=var, func=AF.Ln, bias=eps1)
                    lnr_row = rowp.tile([1, IC], F32, tag="lnr")
                    nc.scalar.mul(out=lnr_row, in_=lnv, mul=-0.5)
                    std_row = rowp.tile([1, IC], F32, tag="stdr")
                    nc.scalar.activation(out=std_row, in_=var, func=AF.Sqrt, bias=eps1)

                    # --- q/k projections (PE stream stays dense; gated small MMs
                    #     are emitted behind big MM blocks) ---
                    pp = {}
                    for nm in ("q", "k"):
                        p = tl_ps.tile([128, IC], F32, tag="tl", name=f"pp_{nm}")
                        for ct in range(NCT):
                            nc.tensor.matmul(p, wps[nm][:, ct, :], xr[:, ct, :],
                                             start=(ct == 0), stop=(ct == NCT - 1))
                        pp[nm] = p
                    sq_q = sqp.tile([128, IC], F32R, tag="sq")
                    nc.scalar.activation(out=sq_q, in_=pp["q"], func=AF.Square)
                    ssq_q = sp_ps.tile([2, IC], F32, tag="sp")
                    nc.tensor.matmul(ssq_q, ones2blk, sq_q, start=True, stop=True)

                    # --- v projection ---
                    vp = tl_ps.tile([128, IC], F32, tag="tl", name="vp")
                    for ct in range(NCT):
                        nc.tensor.matmul(vp, wps["v"][:, ct, :], xr[:, ct, :],
                                         start=(ct == 0), stop=(ct == NCT - 1))

                    # --- finish q l2norm ---
                    rt_q = rowp.tile([2, IC], F32, tag="rowtmp")
                    nc.scalar.activation(out=rt_q, in_=ssq_q, func=AF.Sqrt,
                                         bias=eps2, scale=1.0 / 64.0)
                    rn_q = rowp.tile([2, IC], F32, tag="rowtmp")
                    nc.vector.reciprocal_approx_fast(out=rn_q, in_=rt_q)
                    rnr_q = rowp.tile([2, IC], F32R, tag="rowtmp")
                    nc.vector.tensor_copy(out=rnr_q, in_=rn_q)
                    sr_q = sp_ps.tile([128, IC], F32, tag="sp")
                    nc.tensor.matmul(sr_q, qs2blk, rnr_q, start=True, stop=True)
                    srs_q = srp.tile([128, IC], F32, tag="srs")
                    nc.vector.tensor_copy(out=srs_q, in_=sr_q)
                    nc.vector.tensor_mul(qhat[b][:, isl], pp["q"], srs_q)

                    # --- finish k l2norm ---
                    sq_k = sqp.tile([128, IC], F32R, tag="sq")
                    nc.scalar.activation(out=sq_k, in_=pp["k"], func=AF.Square)
                    ssq_k = sp_ps.tile([2, IC], F32, tag="sp")
                    nc.tensor.matmul(ssq_k, ones2blk, sq_k, start=True, stop=True)
                    rt_k = rowp.tile([2, IC], F32, tag="rowtmp")
                    nc.scalar.activation(out=rt_k, in_=ssq_k, func=AF.Sqrt,
                                         bias=eps2, scale=1.0)
                    rn_k = rowp.tile([2, IC], F32, tag="rowtmp")
                    nc.vector.reciprocal_approx_fast(out=rn_k, in_=rt_k)
                    rnr_k = rowp.tile([2, IC], F32R, tag="rowtmp")
                    nc.vector.tensor_copy(out=rnr_k, in_=rn_k)
                    sr_k = sp_ps.tile([128, IC], F32, tag="sp")
                    nc.tensor.matmul(sr_k, ks2blk, rnr_k, start=True, stop=True)
                    srs_k = srp.tile([128, IC], F32, tag="srs")
                    nc.vector.tensor_copy(out=srs_k, in_=sr_k)
                    nc.vector.tensor_mul(khat[b][:, isl], pp["k"], srs_k)

                    # --- stats transposes: ln(rstd)/std rows -> per-j columns ---
                    for k in range(IC // 128):
                        jt = ic * (IC // 128) + k
                        stl = sp_ps.tile([128, 1], F32, tag="sp")
                        nc.tensor.transpose(stl, lnr_row[:, k * 128:(k + 1) * 128],
                                            ident[0:1, 0:1])
                        nc.vector.tensor_add(ebias[:, b, jt:jt + 1], stl,
                                             kbT[:, b, jt:jt + 1])
                        sts = sp_ps.tile([128, 1], F32, tag="sp")
                        nc.tensor.transpose(sts, std_row[:, k * 128:(k + 1) * 128],
                                            ident[0:1, 0:1])
                        nc.vector.tensor_copy(out=v_all[b][:, jt, 64:65], in_=sts)
                        nc.vector.tensor_copy(out=v_all[b][:, jt, 129:130], in_=sts)

                    # --- v transpose into [j, d] layout ---
                    vsc = sqp.tile([128, IC], BF16, tag="vsc")
                    nc.vector.tensor_copy(out=vsc, in_=vp)
                    for k in range(IC // 128):
                        jt = ic * (IC // 128) + k
                        vt = sp_ps.tile([128, 128], BF16, tag="sp")
                        nc.tensor.transpose(vt, vsc[:, k * 128:(k + 1) * 128], ident_bf)
                        nc.vector.tensor_copy(out=v_all[b][:, jt, 0:64], in_=vt[:, 0:64])
                        nc.vector.tensor_copy(out=v_all[b][:, jt, 65:129], in_=vt[:, 64:128])

                # ================= phase 2 chunk: attention =================
                for b in range(B):
                    avs = [av_ps.tile([65, IC], F32, tag="av", name=f"av{ic}{b}{h}")
                           for h in range(2)]
                    pend = None   # software pipeline: attn@v one tile behind sim
                    for jt in range(jmax):
                        diag_k = jt - (jmax - 4)
                        off = 128 * diag_k if diag_k > 0 else 0
                        for h in range(2):
                            dsl = slice(64 * h, 64 * h + 64)
                            sp = tl_ps.tile([128, IC], F32, tag="tl", name="sim")
                            nc.tensor.matmul(sp[:, off:], ident_bf,
                                             rps[h][:, jt, off:], start=True, stop=False)
                            nc.tensor.matmul(
                                sp[:, off:], khat[b][dsl, jt * 128:(jt + 1) * 128],
                                qhat[b][dsl, ic * IC + off:(ic + 1) * IC],
                                start=False, stop=True)
                            if pend is not None:
                                pE, pjt, ph, poff = pend
                                nc.tensor.matmul(
                                    avs[ph][:, poff:],
                                    v_all[b][:, pjt, 65 * ph:65 * ph + 65],
                                    pE[:, poff:], start=(pjt == 0),
                                    stop=(pjt == jmax - 1))
                            E = ep.tile([128, IC], BF16, tag="E")
                            nc.scalar.activation(out=E[:, off:], in_=sp[:, off:],
                                                 func=AF.Exp, bias=ebias[:, b, jt:jt + 1])
                            pend = (E, jt, h, off)
                    pE, pjt, ph, poff = pend
                    nc.tensor.matmul(
                        avs[ph][:, poff:], v_all[b][:, pjt, 65 * ph:65 * ph + 65],
                        pE[:, poff:], start=(pjt == 0), stop=True)

                    # --- normalize + evacuate ---
                    at_sb = atp.tile([128, IC], BF16, tag="at")
                    for h in range(2):
                        # NB: reciprocal_approx_fast drops PSUM partition-base
                        # offsets (reads partition 0) -- stage the denominator
                        # row through SBUF with tensor_copy first.
                        den_sb = rowp.tile([1, IC], F32, tag="den")
                        nc.vector.tensor_copy(out=den_sb, in_=avs[h][64:65, :])
                        rs_f = rowp.tile([1, IC], F32, tag="rs")
                        nc.vector.reciprocal_approx_fast(out=rs_f, in_=den_sb)
                        rs_r = rowp.tile([1, IC], F32R, tag="rs")
                        nc.vector.tensor_copy(out=rs_r, in_=rs_f)
                        rsb = sp_ps.tile([64, IC], F32, tag="sp")
                        nc.tensor.matmul(rsb, ones64_r, rs_r, start=True, stop=True)
                        rsb_sb = srp.tile([64, IC], F32, tag="rsbs")
                        nc.vector.tensor_copy(out=rsb_sb, in_=rsb)
                        nc.vector.tensor_mul(at_sb[64 * h:64 * h + 64, :],
                                             avs[h][0:64, :], rsb_sb)
                    nc.sync.dma_start(out=at_d.ap()[b, :, isl], in_=at_sb)
    nc.compile()
    return nc


def _build_launch_b():
    import concourse.bass as bass
    import concourse.tile as tile
    from concourse import bacc, mybir

    F32 = mybir.dt.float32
    BF16 = mybir.dt.bfloat16

    nc = bacc.Bacc(None)
    at_d = nc.declare_dram_parameter("a_t", [DIM, IC], BF16, isOutput=False)
    wo_d = nc.declare_dram_parameter("wo", [DIM, DIM], BF16, isOutput=False)
    out_d = nc.declare_dram_parameter("out_rows", [IC, DIM], F32, isOutput=True)

    with tile.TileContext(nc) as tc:
        with tc.tile_pool(name="sb", bufs=1) as sb, \
             tc.tile_pool(name="ob", bufs=2) as ob, \
             tc.tile_pool(name="ps", bufs=1, space="PSUM") as ps:
            warm_row = sb.tile([1, 512], BF16, tag="warm_row")
            nc.vector.memset(warm_row, 0.0)
            warm_col = sb.tile([1, 1], BF16, tag="warm_col")
            nc.vector.memset(warm_col, 0.0)

            a_sb = sb.tile([128, NCT, IC], BF16, tag="a")
            for half in range(2):
                hs = slice(half * (NCT // 2), (half + 1) * (NCT // 2))
                nc.sync.dma_start(
                    out=a_sb[:, hs, :],
                    in_=at_d.ap()[half * 512:(half + 1) * 512, :].rearrange(
                        "(t p) i -> p t i", p=128))
            wo_sb = sb.tile([128, NCT, DIM], BF16, tag="wo")
            for ct in range(NCT):
                nc.sync.dma_start(out=wo_sb[:, ct, :],
                                  in_=wo_d.ap()[ct * 128:(ct + 1) * 128, :])

            pps = [ps.tile([128, 512], F32, tag=f"pp{m}{oc}", name=f"pp{m}{oc}")
                   for m in range(4) for oc in range(2)]
            # warm-up during DMA
            for _ in range(20):
                nc.tensor.matmul(pps[0][0:1, :], warm_col, warm_row,
                                 start=True, stop=True)
            for ct in range(NCT):
                for m in range(4):
                    for oc in range(2):
                        nc.tensor.matmul(
                            pps[2 * m + oc], a_sb[:, ct, m * 128:(m + 1) * 128],
                            wo_sb[:, ct, oc * 512:(oc + 1) * 512],
                            start=(ct == 0), stop=(ct == NCT - 1))
            for m in range(4):
                osb = ob.tile([128, DIM], F32, tag="osb")
                for oc in range(2):
                    nc.vector.tensor_copy(out=osb[:, oc * 512:(oc + 1) * 512],
                                          in_=pps[2 * m + oc])
                nc.sync.dma_start(out=out_d.ap()[m * 128:(m + 1) * 128, :], in_=osb)

    nc.compile()
    return nc


PROFILE = {"enabled": False, "a_ns": None, "b_ns": None}


def _install_profile_hook():
    """Register the axon NTFF profile hook (the image's antenv lacks
    axon_hooks, so run_bass_kernel_spmd(trace=True) would silently skip
    tracing).  Replicates trn_boot's ctypes recipe."""
    import sys, types, ctypes, contextlib

    if "antenv.axon_hooks" in sys.modules:
        return
    lib = ctypes.CDLL("/opt/axon/libaxon_pjrt.so")
    if not hasattr(lib, "axon_start_nrt_profile"):
        return
    lib.axon_start_nrt_profile.argtypes = [ctypes.POINTER(ctypes.c_int64), ctypes.c_size_t]
    lib.axon_start_nrt_profile.restype = ctypes.c_int64
    lib.axon_stop_nrt_profile.argtypes = [ctypes.c_char_p]
    lib.axon_stop_nrt_profile.restype = ctypes.c_int64

    @contextlib.contextmanager
    def _hook(output_dir, device_ids):
        import jax
        jax.devices()
        if device_ids:
            ids = (ctypes.c_int64 * len(device_ids))(*device_ids)
            rc = lib.axon_start_nrt_profile(ids, len(device_ids))
        else:
            rc = lib.axon_start_nrt_profile(None, 0)
        if rc != 0:
            raise RuntimeError(f"axon_start_nrt_profile rc={rc}")
        try:
            yield
        finally:
            n = lib.axon_stop_nrt_profile(str(output_dir).encode())
            print(f"profile: {n} file(s) written to {output_dir}")

    mod = types.ModuleType("antenv.axon_hooks")
    mod.get_axon_ntff_profile_hook = lambda: _hook
    mod.set_axon_ntff_profile_hook = lambda h: None
    sys.modules["antenv.axon_hooks"] = mod

    # avoid the S3 artifact upload inside the trace path
    from concourse import bass_utils
    bass_utils.upload_artifacts = lambda tmpdir: ""


def kernel(x, gamma, Wq, Wkv, q_scale, k_scale, Wo, rel_pos_bias, mask):
    from concourse.bass_utils import run_bass_kernel_spmd
    import ml_dtypes

    BF = ml_dtypes.bfloat16
    x = np.ascontiguousarray(np.asarray(x, dtype=np.float32))
    gamma = np.asarray(gamma, dtype=np.float32)
    Wq = np.asarray(Wq, dtype=np.float32)
    Wkv = np.asarray(Wkv, dtype=np.float32)
    q_scale = np.asarray(q_scale, dtype=np.float32)
    k_scale = np.asarray(k_scale, dtype=np.float32)
    Wo = np.ascontiguousarray(np.asarray(Wo, dtype=np.float32))
    rel_pos_bias = np.asarray(rel_pos_bias, dtype=np.float32)
    mask = np.asarray(mask)

    if PROFILE["enabled"]:
        _install_profile_hook()
    if "a" not in _cache:
        _cache["a"] = _build_launch_a()
    if "b" not in _cache:
        _cache["b"] = _build_launch_b()

    xT = np.ascontiguousarray(x.transpose(0, 2, 1)).astype(BF)
    kb = np.where(mask, 0.0, NEG).astype(np.float32)
    qs2 = np.tile(q_scale, 2).astype(np.float32)
    ks2 = np.tile(k_scale, 2).astype(np.float32)

    # gamma-fold + column-center the projection weights (exact LN mean-sub)
    def prep_w(W):
        Wg = W * gamma[:, None]
        return (Wg - Wg.mean(axis=0, keepdims=True)).astype(BF)

    Wq_p = prep_w(Wq)
    Wk_p = prep_w(Wkv[:, :DIM])
    Wv_p = prep_w(Wkv[:, DIM:])

    # rel_pos bias: transpose to [h, j, i], bake causal mask, bf16
    rpbT = np.ascontiguousarray(rel_pos_bias.transpose(0, 2, 1))
    tri = np.tril(np.ones((N, N), dtype=bool), -1)   # [j, i]: j > i masked
    rpbT[:, tri] = NEG
    rpbT = rpbT.astype(BF)

    in_maps_a = []
    for c in range(NCORES):
        es = slice(EH * c, EH * (c + 1))
        in_maps_a.append({
            "xT": xT,
            "rpbT": np.ascontiguousarray(rpbT[2 * c:2 * c + 2]),
            "wq": np.ascontiguousarray(Wq_p[:, es]),
            "wk": np.ascontiguousarray(Wk_p[:, es]),
            "wv": np.ascontiguousarray(Wv_p[:, es]),
            "qs2": qs2, "ks2": ks2, "kb": kb,
        })
    res_a = run_bass_kernel_spmd(_cache["a"], in_maps_a, list(range(NCORES)),
                                 trace=PROFILE["enabled"])
    if PROFILE["enabled"]:
        PROFILE["a_ns"] = res_a.exec_time_ns

    AT = np.empty((B, DIM, N), BF)
    for c in range(NCORES):
        # [B, 128, N] bf16 (normalized); partitions = [h0 dims 0:64 | h1 dims 0:64]
        AT[:, EH * c:EH * (c + 1), :] = res_a.results[c]["at_out"]

    Wo_bf = Wo.astype(BF)
    in_maps_b = []
    for c in range(NCORES):
        bi, icn = c // NIC, c % NIC
        in_maps_b.append({
            "a_t": np.ascontiguousarray(AT[bi][:, icn * IC:(icn + 1) * IC]),
            "wo": Wo_bf,
        })
    res_b = run_bass_kernel_spmd(_cache["b"], in_maps_b, list(range(NCORES)),
                                 trace=PROFILE["enabled"])
    if PROFILE["enabled"]:
        PROFILE["b_ns"] = res_b.exec_time_ns

    out = np.empty((B, N, DIM), np.float32)
    for c in range(NCORES):
        bi, icn = c // NIC, c % NIC
        out[bi, icn * IC:(icn + 1) * IC, :] = res_b.results[c]["out_rows"]
    return out
